# revision 1
# baseline (speedup 1.0000x reference)
"""Causal self-attention with RoPE, tensor-parallel over 8 NeuronCores.

Problem shapes: x [2, 2048, 2048], w_qkv [6144, 2048], w_out [2048, 2048],
H=16 heads, dh=128. Core c owns heads {2c, 2c+1}.

Per-core plan (all layouts chosen so no on-device transposes are needed):
  - host passes xT [B, D, L] plus per-core transposed weight shards
  - Q^T/K^T projection: psum[dh, tok] = W^T-tile (stationary) x xT (moving);
    V is projected straight into natural [tok, dh] layout by flipping the
    operands (x-tile token slices stationary, wvT moving)
  - RoPE applied during PSUM->SBUF copyback via crossed-base-partition
    DVE ops (half-swap without any shuffle instruction)
  - scores^T[k, q] = K^T-tile (stationary) x Q^T (moving); exp on ACT with
    the 1/sqrt(dh) scale folded in; causal mask = multiply by 0/1 tiles on
    the (only partially-valid) diagonal blocks; fully-masked blocks skipped
  - unnormalized out^T[dh, q] accumulates V-tile x E^T; the softmax
    denominator accumulates ones^T x E^T in the same pass
  - normalization: reciprocal of the [1, 512] sums row, broadcast across
    partitions with gpsimd partition_broadcast (attn ucode library),
    multiplied in during the attention-out copyback; w_out for chunk qc is
    emitted after attention for qc+1 so the PE never waits on that chain
  - w_out partial[tok, e] = attnout^T-tile (stationary) x woT (moving),
    summed over this core's heads in PSUM; host sums the 8 partials

Matmuls run in float32r (full PE rate for free dim >= 256, ~1.5e-4 rel
err per K=128 contraction measured on hw).
"""

import numpy as np

import concourse.bass as bass
import concourse.mybir as mybir
import concourse.tile as tile
from concourse import bacc, library_config
from concourse.bass_utils import run_bass_kernel_spmd

B, L, D, H = 2, 2048, 2048, 16
DH = D // H  # 128
NCORES = 8
HPC = H // NCORES  # heads per core
ROPE_BASE = 10000.0
SCALE = 1.0 / float(np.sqrt(np.float32(DH)))

TOKC = 256  # token chunk width in the QKV projection phase
NCHUNK = L // TOKC  # 8
QC = 512  # q chunk width in the attention phase
NQC = L // QC  # 4
KT = L // 128  # 16 k tiles per sequence
KD = D // 128  # 16 contraction chunks for the projections

F32 = mybir.dt.float32
F32R = mybir.dt.float32r
# pool depths (tunable for experiments)
POOL = {"xtp": 2, "tmps": 2, "esb": 5, "bcp": 2, "attn": 2, "outp": 8, "psA": 4}
AF = mybir.ActivationFunctionType
ALU = mybir.AluOpType


def _body(nc, tc, aps, phases=("qkv", "attn", "wout")):
    xt, wq, wk, wv, wo, cs, mk, out = aps
    with (
        tc.tile_pool(name="const", bufs=1) as const,
        tc.tile_pool(name="xtp", bufs=POOL["xtp"]) as xtp,
        tc.tile_pool(name="qkv", bufs=1) as qkvp,
        tc.tile_pool(name="tmps", bufs=POOL["tmps"]) as tmps,
        tc.tile_pool(name="esb", bufs=POOL["esb"]) as esbp,
        tc.tile_pool(name="bcp", bufs=POOL["bcp"]) as bcp,
        tc.tile_pool(name="attn", bufs=POOL["attn"]) as attnp,
        tc.tile_pool(name="outp", bufs=POOL["outp"]) as outp,
        tc.tile_pool(name="psA", bufs=POOL["psA"], space="PSUM") as psA,
        tc.tile_pool(name="psO", bufs=2, space="PSUM") as psO,
        tc.tile_pool(name="psS", bufs=2, space="PSUM") as psS,
    ):
        # ---- constants ----
        wq_sb = const.tile([128, KD, HPC * DH], F32R, name="wq_sb")
        wk_sb = const.tile([128, KD, HPC * DH], F32R, name="wk_sb")
        wv_sb = const.tile([128, KD, HPC * DH], F32R, name="wv_sb")
        wo_sb = const.tile([128, HPC, D], F32R, name="wo_sb")
        # tbl packs cos (partitions 0:64) and sin (partitions 64:128)
        tbl_sb = const.tile([128, L], F32, name="tbl_sb")
        mk_sb = const.tile([128, 4, QC], F32R, name="mk_sb")
        def load_chunk(b, c):
            c0 = c * TOKC
            xtile = xtp.tile([128, KD, TOKC], F32R, name="xtile")
            src = xt[b, :, c0:c0 + TOKC].rearrange("(ko p) n -> p ko n", p=128)
            # two half-loads: matmuls on k<8 start as soon as half 0 lands
            nc.sync.dma_start(xtile[:, 0:KD // 2], src[:, 0:KD // 2])
            nc.sync.dma_start(xtile[:, KD // 2:], src[:, KD // 2:])
            return xtile

        # first x chunk + first weight ahead of everything else; wo last
        # (first needed ~150us in) so the first QKV matmul starts early
        xtile00 = load_chunk(0, 0) if "qkv" in phases else None
        for dst, src in ((wq_sb, wq), (tbl_sb, cs), (mk_sb, mk),
                         (wk_sb, wk), (wv_sb, wv), (wo_sb, wo)):
            nc.sync.dma_start(dst, src)
        ones_f32 = const.tile([128, 1], F32, name="ones_f32")
        nc.vector.memset(ones_f32, 1.0)
        ones_col = const.tile([128, 1], F32R, name="ones_col")
        nc.vector.tensor_copy(ones_col, ones_f32)

        for b in range(B):
            # ---- QKV projection + RoPE for batch b ----
            qrot = [qkvp.tile([128, L], F32R, name=f"qrot{h}") for h in range(HPC)]
            krot = [qkvp.tile([128, L], F32R, name=f"krot{h}") for h in range(HPC)]
            vnat = [qkvp.tile([128, KT, 128], F32R, name=f"vnat{h}")
                    for h in range(HPC)]
            for c in range(NCHUNK if "qkv" in phases else 0):
                c0 = c * TOKC
                xtile = xtile00 if (b == 0 and c == 0) else load_chunk(b, c)
                for w_sb, dsts in ((wq_sb, qrot), (wk_sb, krot)):
                    for h in range(HPC):
                        ps = psA.tile([128, TOKC], F32, name="ps_proj", tag="psA")
                        for k in range(KD):
                            nc.tensor.matmul(
                                ps, w_sb[:, k, h * DH:(h + 1) * DH], xtile[:, k, :],
                                start=(k == 0), stop=(k == KD - 1),
                            )
                        if "nocopy" in phases:
                            continue
                        # RoPE copyback: dst = psum*cos +/- swap(psum)*sin
                        # tbl[0:64] = cosT, tbl[64:128] = sinT (64 freq rows)
                        cseg = tbl_sb[0:64, c0:c0 + TOKC]
                        sseg = tbl_sb[64:128, c0:c0 + TOKC]
                        t = tmps.tile([128, TOKC], F32, name="rope_t")
                        a = tmps.tile([128, TOKC], F32, name="rope_a")
                        nc.vector.tensor_tensor(t[0:64], ps[64:128], sseg, ALU.mult)
                        nc.vector.tensor_tensor(t[64:128], ps[0:64], sseg, ALU.mult)
                        nc.vector.tensor_tensor(a[0:64], ps[0:64], cseg, ALU.mult)
                        nc.vector.tensor_tensor(a[64:128], ps[64:128], cseg, ALU.mult)
                        dst = dsts[h]
                        nc.vector.tensor_tensor(
                            dst[0:64, c0:c0 + TOKC], a[0:64], t[0:64], ALU.subtract)
                        nc.vector.tensor_tensor(
                            dst[64:128, c0:c0 + TOKC], a[64:128], t[64:128], ALU.add)
                # V in natural [tok, dh] layout directly: x-tile token slices
                # are the stationary, both heads share one moving pass
                for s in range(TOKC // 128):
                    psv = psA.tile([128, HPC * DH], F32, name="ps_v", tag="psA")
                    for k in range(KD):
                        nc.tensor.matmul(
                            psv, xtile[:, k, s * 128:(s + 1) * 128], wv_sb[:, k, :],
                            start=(k == 0), stop=(k == KD - 1),
                        )
                    if "nocopy" in phases:
                        continue
                    kti = (c0 // 128) + s
                    for h in range(HPC):
                        nc.vector.tensor_copy(
                            vnat[h][:, kti, :], psv[:, h * DH:(h + 1) * DH])

            # ---- attention + output projection, per 512-token q chunk ----
            # w_out for chunk qc is emitted after attention for qc+1 so the
            # PE never waits on the softmax-normalize chain.
            def emit_wout(attn_sb, q0):
                for mt in range(QC // 128 if "wout" in phases else 0):
                    t0 = q0 + mt * 128
                    for ec in range(D // 512):
                        psw = psA.tile([128, 512], F32, name="ps_w", tag="psA")
                        for h in range(HPC):
                            nc.tensor.matmul(
                                psw, attn_sb[h][:, mt * 128:(mt + 1) * 128],
                                wo_sb[:, h, ec * 512:(ec + 1) * 512],
                                start=(h == 0), stop=(h == HPC - 1),
                            )
                        ob = outp.tile([128, 512], F32, name="out_sb")
                        if (mt + ec) % 2 == 0:
                            nc.scalar.copy(ob, psw)
                        else:
                            nc.vector.tensor_copy(ob, psw)
                        eng = nc.scalar if (mt + ec) % 2 == 0 else nc.sync
                        eng.dma_start(
                            out[b, t0:t0 + 128, ec * 512:(ec + 1) * 512], ob)

            pend = None
            for qc in range(NQC if "attn" in phases else 0):
                q0 = qc * QC
                attn_sb = []
                for h in range(HPC):
                    pso = psO.tile([128, QC], F32, name="ps_out")
                    pss = psS.tile([1, QC], F32, name="ps_sum")
                    nkt = (qc + 1) * (QC // 128)
                    for kt in range(nkt):
                        psc = psA.tile([128, QC], F32, name="ps_sc", tag="psA")
                        nc.tensor.matmul(
                            psc, krot[h][:, kt * 128:(kt + 1) * 128],
                            qrot[h][:, q0:q0 + QC], start=True, stop=True,
                        )
                        e = esbp.tile([128, QC], F32R, name="e_sb")
                        if "noexp" in phases:
                            nc.scalar.copy(e, psc)
                        else:
                            nc.scalar.activation(e, psc, AF.Exp, scale=SCALE)
                        diag = kt - qc * (QC // 128)
                        if diag >= 0 and "nomask" not in phases:
                            nc.vector.tensor_tensor(e, e, mk_sb[:, diag, :], ALU.mult)
                        nc.tensor.matmul(pso, vnat[h][:, kt, :], e,
                                         start=(kt == 0), stop=(kt == nkt - 1))
                        if "nosum" not in phases:
                            nc.tensor.matmul(pss, ones_col, e,
                                             start=(kt == 0), stop=(kt == nkt - 1))
                    att = attnp.tile([128, QC], F32R, name=f"att{h}")
                    if "nosum" in phases:
                        nc.vector.tensor_copy(att, pso)
                    else:
                        rec = tmps.tile([1, QC], F32, name="recip")
                        nc.vector.reciprocal(rec, pss)
                        bc = bcp.tile([128, QC], F32, name="bc_sb")
                        nc.gpsimd.partition_broadcast(bc, rec)
                        nc.vector.tensor_tensor(att, pso, bc, ALU.mult)
                    attn_sb.append(att)
                if pend is not None:
                    emit_wout(*pend)
                pend = (attn_sb, q0)
            if pend is not None:
                emit_wout(*pend)


def build_kernel(timing=False, loop_n=0, phases=("qkv", "attn", "wout")):
    nc = bacc.Bacc(
        "TRN2",
        target_bir_lowering=False,
        debug=False,
        enable_asserts=False,
        num_devices=NCORES,
    )
    xt = nc.dram_tensor("xt", [B, D, L], F32R, kind="ExternalInput").ap()
    wq = nc.dram_tensor("wq", [128, KD, HPC * DH], F32R, kind="ExternalInput").ap()
    wk = nc.dram_tensor("wk", [128, KD, HPC * DH], F32R, kind="ExternalInput").ap()
    wv = nc.dram_tensor("wv", [128, KD, HPC * DH], F32R, kind="ExternalInput").ap()
    wo = nc.dram_tensor("wo", [128, HPC, D], F32R, kind="ExternalInput").ap()
    cs = nc.dram_tensor("cs", [128, L], F32, kind="ExternalInput").ap()
    mk = nc.dram_tensor("mk", [128, 4, QC], F32R, kind="ExternalInput").ap()
    out_kind = "Internal" if timing else "ExternalOutput"
    out = nc.dram_tensor("out", [B, L, D], F32, kind=out_kind).ap()
    done = None
    if timing:
        done = nc.dram_tensor("done", [1, 4], F32, kind="ExternalOutput").ap()

    nc.gpsimd.load_library(library_config.attn)
    aps = (xt, wq, wk, wv, wo, cs, mk, out)
    with tile.TileContext(nc) as tc:
        if loop_n:
            with tc.For_i(0, loop_n, 1):
                _body(nc, tc, aps, phases)
        else:
            _body(nc, tc, aps, phases)
        if timing:
            # tiny output so the executable has an ExternalOutput; depends on
            # one real out tile via a DRAM->DRAM DMA of the last row.
            nc.sync.dma_start(done, out[B - 1, L - 1:L, 0:4])
    nc.compile()
    return nc


def _rope_tables():
    inv_freq = (1.0 / (ROPE_BASE ** (np.arange(0, DH, 2, dtype=np.float32) / DH))
                ).astype(np.float32)
    freqs = (np.arange(L, dtype=np.float32)[:, None] * inv_freq[None, :]
             ).astype(np.float32)  # [L, 64]
    cos_t = np.cos(freqs).astype(np.float32).T  # [64, L]
    sin_t = np.sin(freqs).astype(np.float32).T
    tbl = np.concatenate([cos_t, sin_t], axis=0)  # [128, L]
    return np.ascontiguousarray(tbl)


def _host_inputs(x, w_qkv, w_out):
    xt = np.ascontiguousarray(np.transpose(x, (0, 2, 1)))  # [B, D, L]
    cs = _rope_tables()
    p = np.arange(128)[:, None]
    f = np.arange(QC)[None, :]
    mk = np.stack(
        [((bi * 128 + p) <= f).astype(np.float32) for bi in range(4)], axis=1
    )  # [128, 4, 512]
    mk = np.ascontiguousarray(mk)

    def wtile(wT):  # [D, M] -> [128, D//128, M]
        return np.ascontiguousarray(
            wT.reshape(KD, 128, wT.shape[1]).transpose(1, 0, 2))

    in_maps = []
    for c in range(NCORES):
        r0 = c * HPC * DH
        r1 = r0 + HPC * DH
        wq_c = wtile(np.ascontiguousarray(w_qkv[r0:r1, :].T))
        wk_c = wtile(np.ascontiguousarray(w_qkv[D + r0:D + r1, :].T))
        wv_c = wtile(np.ascontiguousarray(w_qkv[2 * D + r0:2 * D + r1, :].T))
        wo_c = np.ascontiguousarray(
            w_out[:, r0:r1].T.reshape(HPC, 128, D).transpose(1, 0, 2))
        in_maps.append({
            "xt": xt, "wq": wq_c, "wk": wk_c, "wv": wv_c, "wo": wo_c,
            "cs": cs, "mk": mk,
        })
    return in_maps


_NC_CACHE = []


def _get_nc():
    if not _NC_CACHE:
        _NC_CACHE.append(build_kernel())
    return _NC_CACHE[0]


def kernel(x, w_qkv, w_out):
    x = np.asarray(x, dtype=np.float32)
    w_qkv = np.asarray(w_qkv, dtype=np.float32)
    w_out = np.asarray(w_out, dtype=np.float32)
    nc = _get_nc()
    in_maps = _host_inputs(x, w_qkv, w_out)
    res = run_bass_kernel_spmd(nc, in_maps, core_ids=list(range(NCORES)))
    acc = res.results[0]["out"].astype(np.float32)
    for c in range(1, NCORES):
        acc += res.results[c]["out"]
    return acc



# revision 6
# speedup vs baseline: 1.0884x; 1.0884x over previous
"""Causal self-attention with RoPE on 8 NeuronCores.

Problem shapes: x [2, 2048, 2048], w_qkv [6144, 2048], w_out [2048, 2048],
H=16 heads, dh=128.

Sharding: data-parallel over batch x tensor-parallel over heads.  Core c
owns batch c//4 and heads {4g..4g+3} with g = c%4.  Each core computes a
[2048, 2048] partial of its batch's output (w_out rows for its heads);
the host sums 4 partials per batch.  Per-core DMA: x 8.4MB bf16 in,
weights 8MB bf16, out 16.8MB f32 -- about half the all-TP layout.

All matmuls run in bf16 (1 row/cycle on PE, f32 PSUM accumulation;
rel err ~0.5% << the 2e-2 gate).  Per-core plan:

  - Q^T/K^T projection: psum[dh, tok] = w-tile (stationary) x xT (moving,
    512-token chunks); V projected into natural [tok, dh] layout by
    flipping operands (x token-slices stationary, wv moving, all 4 heads
    in one 512-wide moving pass).
  - RoPE during PSUM->SBUF copyback, 4 DVE ops per tile using
    sign-folded tables (S = [-sin; +sin], C = [cos; cos]):
    t = swap(ps) * S (2 half ops), dst = ps * C + t (2 full ops).
  - Attention per 512-token q chunk, k tiles processed in PAIRS sharing
    one [128, 1024] PSUM tile so a single exp activation covers both
    (ACT per pair 1147ns < PE per pair ~1280ns).  The attn-out/denom
    matmuls for pair p are emitted after the scores matmuls of pair p+2
    (software pipelining, 2-pair lag) so PE never waits on ACT.
  - Causal mask: multiply e by 0/1 tiles on diagonal blocks (bf16 DVE).
  - Softmax denominator accumulates ones^T x e on PE alongside attn-out;
    reciprocal via the 1-instruction approx-fast DVE op (~51 ULP),
    broadcast across partitions with gpsimd, multiplied in during the
    attn-out copyback.
  - w_out for chunk qc is emitted after attention for qc+1; projections
    for chunk c+1 interleave with attention for chunk c.
"""

import numpy as np

import concourse.bass as bass
import concourse.mybir as mybir
import concourse.tile as tile
from concourse import bacc, library_config
from concourse.bass_utils import run_bass_kernel_spmd

B, L, D, H = 2, 2048, 2048, 16
DH = D // H  # 128
NCORES = 8
NGRP = 4  # head groups (cores per batch)
HPC = H // NGRP  # 4 heads per core
ROPE_BASE = 10000.0
SCALE = 1.0 / float(np.sqrt(np.float32(DH)))

TOKC = 512  # token chunk width in the QKV projection phase
NCHUNK = L // TOKC  # 4
QC = 512  # q chunk width in the attention phase (== TOKC)
KT = L // 128  # 16 k tiles per sequence
KD = D // 128  # 16 contraction chunks for the projections

F32 = mybir.dt.float32
BF16 = mybir.dt.bfloat16
AF = mybir.ActivationFunctionType
ALU = mybir.AluOpType


def _body(nc, tc, aps, phases=("qkv", "attn", "wout")):
    xt, wq, wk, wv, wo, cc, ss, mk, out = aps
    with (
        tc.tile_pool(name="const", bufs=1) as const,
        tc.tile_pool(name="xtp", bufs=2) as xtp,
        tc.tile_pool(name="qkv", bufs=1) as qkvp,
        tc.tile_pool(name="rope", bufs=1) as ropep,
        tc.tile_pool(name="esb", bufs=3) as esbp,
        tc.tile_pool(name="recp", bufs=1) as recp,
        tc.tile_pool(name="bcp", bufs=2) as bcp,
        tc.tile_pool(name="attn", bufs=2) as attnp,
        tc.tile_pool(name="outp", bufs=4) as outp,
        tc.tile_pool(name="psA", bufs=2, space="PSUM") as psA,
        tc.tile_pool(name="psO", bufs=2, space="PSUM") as psO,
        tc.tile_pool(name="psS", bufs=2, space="PSUM") as psS,
    ):
        # ---- constants ----
        wq_sb = const.tile([128, KD, HPC * DH], BF16, name="wq_sb")
        wk_sb = const.tile([128, KD, HPC * DH], BF16, name="wk_sb")
        wv_sb = const.tile([128, KD, HPC * DH], BF16, name="wv_sb")
        wo_sb = const.tile([128, HPC, D], BF16, name="wo_sb")
        cc_sb = const.tile([128, L], F32, name="cc_sb")  # [cos; cos]
        ss_sb = const.tile([128, L], F32, name="ss_sb")  # [-sin; +sin]
        mk_sb = const.tile([128, 4, QC], BF16, name="mk_sb")

        def load_chunk(c):
            c0 = c * TOKC
            xtile = xtp.tile([128, KD, TOKC], BF16, name="xtile")
            src = xt[:, c0:c0 + TOKC].rearrange("(ko p) n -> p ko n", p=128)
            # two half-loads: matmuls on k<8 start as soon as half 0 lands
            nc.sync.dma_start(xtile[:, 0:KD // 2], src[:, 0:KD // 2])
            nc.sync.dma_start(xtile[:, KD // 2:], src[:, KD // 2:])
            return xtile

        # first x chunk + first weight ahead of everything else; wo last
        xtile0 = load_chunk(0) if "qkv" in phases else None
        for dst, src in ((wq_sb, wq), (cc_sb, cc), (ss_sb, ss), (mk_sb, mk),
                         (wk_sb, wk), (wv_sb, wv), (wo_sb, wo)):
            nc.sync.dma_start(dst, src)
        ones_f32 = const.tile([128, 1], F32, name="ones_f32")
        nc.vector.memset(ones_f32, 1.0)
        ones_col = const.tile([128, 1], BF16, name="ones_col")
        nc.vector.tensor_copy(ones_col, ones_f32)

        qrot = [qkvp.tile([128, L], BF16, name=f"qrot{h}") for h in range(HPC)]
        krot = [qkvp.tile([128, L], BF16, name=f"krot{h}") for h in range(HPC)]
        vnat = qkvp.tile([128, KT, HPC * DH], BF16, name="vnat")

        def proj_chunk(c):
            c0 = c * TOKC
            xtile = xtile0 if c == 0 else load_chunk(c)
            cseg = cc_sb[:, c0:c0 + TOKC]
            sseg = ss_sb[:, c0:c0 + TOKC]
            for w_sb, dsts in ((wq_sb, qrot), (wk_sb, krot)):
                for h in range(HPC):
                    ps = psA.tile([128, TOKC], F32, name="ps_proj", tag="psA")
                    for k in range(KD):
                        nc.tensor.matmul(
                            ps, w_sb[:, k, h * DH:(h + 1) * DH], xtile[:, k, :],
                            start=(k == 0), stop=(k == KD - 1),
                        )
                    if "nocopy" in phases:
                        continue
                    # RoPE copyback: dst = ps*C + swap(ps)*S
                    t = ropep.tile([128, TOKC], F32, name="rope_t")
                    a = ropep.tile([128, TOKC], F32, name="rope_a")
                    nc.vector.tensor_tensor(t[0:64], ps[64:128], sseg[0:64], ALU.mult)
                    nc.vector.tensor_tensor(t[64:128], ps[0:64], sseg[64:128], ALU.mult)
                    nc.vector.tensor_tensor(a, ps, cseg, ALU.mult)
                    nc.vector.tensor_tensor(
                        dsts[h][:, c0:c0 + TOKC], a, t, ALU.add)
            # V in natural [tok, dh] layout: x token slices stationary, all
            # 4 heads share one 512-wide moving pass
            for s in range(TOKC // 128):
                psv = psA.tile([128, HPC * DH], F32, name="ps_v", tag="psA")
                for k in range(KD):
                    nc.tensor.matmul(
                        psv, xtile[:, k, s * 128:(s + 1) * 128], wv_sb[:, k, :],
                        start=(k == 0), stop=(k == KD - 1),
                    )
                if "nocopy" in phases:
                    continue
                kti = (c0 // 128) + s
                nc.scalar.copy(vnat[:, kti, :], psv)

        def emit_wout(attn_sb, q0):
            for mt in range(QC // 128):
                t0 = q0 + mt * 128
                for ec in range(D // 512):
                    psw = psA.tile([128, 512], F32, name="ps_w", tag="psA")
                    for h in range(HPC):
                        nc.tensor.matmul(
                            psw, attn_sb[h][:, mt * 128:(mt + 1) * 128],
                            wo_sb[:, h, ec * 512:(ec + 1) * 512],
                            start=(h == 0), stop=(h == HPC - 1),
                        )
                    ob = outp.tile([128, 512], F32, name="out_sb")
                    if (mt + ec) % 2 == 0:
                        nc.scalar.copy(ob, psw)
                    else:
                        nc.vector.tensor_copy(ob, psw)
                    eng = nc.scalar if (mt + ec) % 2 == 0 else nc.sync
                    eng.dma_start(
                        out[t0:t0 + 128, ec * 512:(ec + 1) * 512], ob)

        def attn_chunk(qc):
            q0 = qc * QC
            attn_sb = []
            for h in range(HPC):
                pso = psO.tile([128, QC], F32, name="ps_out")
                pss = psS.tile([1, QC], F32, name="ps_sum")
                nkt = (qc + 1) * (QC // 128)
                npair = nkt // 2
                epairs = [None] * npair

                def emit_attnout(q):
                    e = epairs[q]
                    for half in range(2):
                        kt = 2 * q + half
                        eh = e[:, half * QC:(half + 1) * QC]
                        nc.tensor.matmul(
                            pso, vnat[:, kt, h * DH:(h + 1) * DH], eh,
                            start=(kt == 0), stop=(kt == nkt - 1))
                        if "nosum" not in phases:
                            nc.tensor.matmul(
                                pss, ones_col, eh,
                                start=(kt == 0), stop=(kt == nkt - 1))

                for p in range(npair + 2):
                    if p < npair:
                        pp = psA.tile([128, 2 * QC], F32, name="ps_sc", tag="psA")
                        for half in range(2):
                            kt = 2 * p + half
                            nc.tensor.matmul(
                                pp[:, half * QC:(half + 1) * QC],
                                krot[h][:, kt * 128:(kt + 1) * 128],
                                qrot[h][:, q0:q0 + QC], start=True, stop=True,
                            )
                        e = esbp.tile([128, 2 * QC], BF16, name="e_sb")
                        if "noexp" in phases:
                            nc.scalar.copy(e, pp)
                        else:
                            nc.scalar.activation(e, pp, AF.Exp, scale=SCALE)
                        if "nomask" not in phases:
                            for half in range(2):
                                diag = 2 * p + half - qc * (QC // 128)
                                if diag >= 0:
                                    eh = e[:, half * QC:(half + 1) * QC]
                                    nc.vector.tensor_tensor(
                                        eh, eh, mk_sb[:, diag, :], ALU.mult)
                        epairs[p] = e
                    if p >= 2:
                        emit_attnout(p - 2)

                att = attnp.tile([128, QC], BF16, name=f"att{h}")
                if "nosum" in phases:
                    nc.vector.tensor_copy(att, pso)
                else:
                    rec = recp.tile([1, QC], F32, name="recip")
                    nc.vector.reciprocal_approx_fast(rec, pss)
                    bc = bcp.tile([128, QC], F32, name="bc_sb")
                    nc.gpsimd.partition_broadcast(bc, rec)
                    nc.vector.tensor_tensor(att, pso, bc, ALU.mult)
                attn_sb.append(att)
            return attn_sb

        pend = None
        for c in range(NCHUNK):
            if "qkv" in phases:
                proj_chunk(c)
            if "attn" in phases:
                attn_sb = attn_chunk(c)
                if pend is not None and "wout" in phases:
                    emit_wout(*pend)
                pend = (attn_sb, c * QC)
        if pend is not None and "wout" in phases:
            emit_wout(*pend)


def build_kernel(timing=False, loop_n=0, phases=("qkv", "attn", "wout")):
    nc = bacc.Bacc(
        "TRN2",
        target_bir_lowering=False,
        debug=False,
        enable_asserts=False,
        num_devices=NCORES,
    )
    xt = nc.dram_tensor("xt", [D, L], BF16, kind="ExternalInput").ap()
    wq = nc.dram_tensor("wq", [128, KD, HPC * DH], BF16, kind="ExternalInput").ap()
    wk = nc.dram_tensor("wk", [128, KD, HPC * DH], BF16, kind="ExternalInput").ap()
    wv = nc.dram_tensor("wv", [128, KD, HPC * DH], BF16, kind="ExternalInput").ap()
    wo = nc.dram_tensor("wo", [128, HPC, D], BF16, kind="ExternalInput").ap()
    cc = nc.dram_tensor("cc", [128, L], F32, kind="ExternalInput").ap()
    ss = nc.dram_tensor("ss", [128, L], F32, kind="ExternalInput").ap()
    mk = nc.dram_tensor("mk", [128, 4, QC], BF16, kind="ExternalInput").ap()
    out_kind = "Internal" if timing else "ExternalOutput"
    out = nc.dram_tensor("out", [L, D], F32, kind=out_kind).ap()
    done = None
    if timing:
        done = nc.dram_tensor("done", [1, 4], F32, kind="ExternalOutput").ap()

    nc.gpsimd.load_library(library_config.attn)
    aps = (xt, wq, wk, wv, wo, cc, ss, mk, out)
    with tile.TileContext(nc) as tc:
        if loop_n:
            with tc.For_i(0, loop_n, 1):
                _body(nc, tc, aps, phases)
        else:
            _body(nc, tc, aps, phases)
        if timing:
            # tiny output so the executable has an ExternalOutput; depends on
            # one real out tile via a DRAM->DRAM DMA of the last row.
            nc.sync.dma_start(done, out[L - 1:L, 0:4])
    nc.compile()
    return nc


def _rope_tables():
    inv_freq = (1.0 / (ROPE_BASE ** (np.arange(0, DH, 2, dtype=np.float32) / DH))
                ).astype(np.float32)
    freqs = (np.arange(L, dtype=np.float32)[:, None] * inv_freq[None, :]
             ).astype(np.float32)  # [L, 64]
    cos_t = np.cos(freqs).astype(np.float32).T  # [64, L]
    sin_t = np.sin(freqs).astype(np.float32).T
    cc = np.concatenate([cos_t, cos_t], axis=0)  # [128, L]
    ss = np.concatenate([-sin_t, sin_t], axis=0)
    return np.ascontiguousarray(cc), np.ascontiguousarray(ss)


def _host_inputs(x, w_qkv, w_out):
    bf16 = mybir.dt.np(BF16)
    cc, ss = _rope_tables()
    p = np.arange(128)[:, None]
    f = np.arange(QC)[None, :]
    mk = np.stack(
        [((bi * 128 + p) <= f).astype(np.float32) for bi in range(4)], axis=1
    ).astype(bf16)  # [128, 4, 512]
    mk = np.ascontiguousarray(mk)

    def wtile(wT):  # [D, M] f32 -> [128, D//128, M] bf16
        return np.ascontiguousarray(
            wT.reshape(KD, 128, wT.shape[1]).transpose(1, 0, 2)).astype(bf16)

    xts = [
        np.ascontiguousarray(x[b].T).astype(bf16)  # [D, L]
        for b in range(B)
    ]
    in_maps = []
    for c in range(NCORES):
        b, g = divmod(c, NGRP)
        r0 = g * HPC * DH
        r1 = r0 + HPC * DH
        wq_c = wtile(np.ascontiguousarray(w_qkv[r0:r1, :].T))
        wk_c = wtile(np.ascontiguousarray(w_qkv[D + r0:D + r1, :].T))
        wv_c = wtile(np.ascontiguousarray(w_qkv[2 * D + r0:2 * D + r1, :].T))
        wo_c = np.ascontiguousarray(
            w_out[:, r0:r1].T.reshape(HPC, 128, D).transpose(1, 0, 2)
        ).astype(bf16)
        in_maps.append({
            "xt": xts[b], "wq": wq_c, "wk": wk_c, "wv": wv_c, "wo": wo_c,
            "cc": cc, "ss": ss, "mk": mk,
        })
    return in_maps


_NC_CACHE = []


def _get_nc():
    if not _NC_CACHE:
        _NC_CACHE.append(build_kernel())
    return _NC_CACHE[0]


def kernel(x, w_qkv, w_out):
    x = np.asarray(x, dtype=np.float32)
    w_qkv = np.asarray(w_qkv, dtype=np.float32)
    w_out = np.asarray(w_out, dtype=np.float32)
    nc = _get_nc()
    in_maps = _host_inputs(x, w_qkv, w_out)
    res = run_bass_kernel_spmd(nc, in_maps, core_ids=list(range(NCORES)))
    out = np.zeros((B, L, D), dtype=np.float32)
    for c in range(NCORES):
        out[c // NGRP] += res.results[c]["out"]
    return out


# revision 9
# speedup vs baseline: 1.1415x; 1.0488x over previous
"""Causal self-attention with RoPE on 8 NeuronCores.

Problem shapes: x [2, 2048, 2048], w_qkv [6144, 2048], w_out [2048, 2048],
H=16 heads, dh=128.

Sharding: data-parallel over batch x tensor-parallel over heads.  Core c
owns batch c//4 and heads {4g..4g+3} with g = c%4.  Each core computes a
[2048, 2048] partial of its batch's output (w_out rows for its heads);
the host sums 4 partials per batch.  Per-core DMA: x 8.4MB bf16 in,
weights 8MB bf16, out 16.8MB f32 -- about half the all-TP layout.

All matmuls run in bf16 (1 row/cycle on PE, f32 PSUM accumulation;
rel err ~0.5% << the 2e-2 gate).  Per-core plan:

  - Q^T/K^T projection: psum[dh, tok] = w-tile (stationary) x xT (moving,
    512-token chunks); V projected into natural [tok, dh] layout by
    flipping operands (x token-slices stationary, wv moving, all 4 heads
    in one 512-wide moving pass).
  - RoPE during PSUM->SBUF copyback, 4 DVE ops per tile using
    sign-folded tables (S = [-sin; +sin], C = [cos; cos]):
    t = swap(ps) * S (2 half ops), dst = ps * C + t (2 full ops).
  - Attention per 512-token q chunk, k tiles processed in PAIRS sharing
    one [128, 1024] PSUM tile so a single exp activation covers both
    (ACT per pair 1147ns < PE per pair ~1280ns).  The attn-out/denom
    matmuls for pair p are emitted after the scores matmuls of pair p+2
    (software pipelining, 2-pair lag) so PE never waits on ACT.
  - Causal mask: multiply e by 0/1 tiles on diagonal blocks (bf16 DVE).
  - Softmax denominator accumulates ones^T x e on PE alongside attn-out;
    reciprocal via the 1-instruction approx-fast DVE op (~51 ULP),
    broadcast across partitions with gpsimd, multiplied in during the
    attn-out copyback.
  - w_out for chunk qc is emitted after attention for qc+1; projections
    for chunk c+1 interleave with attention for chunk c.
"""

import numpy as np

import concourse.bass as bass
import concourse.mybir as mybir
import concourse.tile as tile
from concourse import bacc, library_config
from concourse.bass_utils import run_bass_kernel_spmd

B, L, D, H = 2, 2048, 2048, 16
DH = D // H  # 128
NCORES = 8
NGRP = 4  # head groups (cores per batch)
HPC = H // NGRP  # 4 heads per core
ROPE_BASE = 10000.0
SCALE = 1.0 / float(np.sqrt(np.float32(DH)))

TOKC = 512  # token chunk width in the QKV projection phase
NCHUNK = L // TOKC  # 4
QC = 512  # q chunk width in the attention phase (== TOKC)
KT = L // 128  # 16 k tiles per sequence
KD = D // 128  # 16 contraction chunks for the projections

F32 = mybir.dt.float32
BF16 = mybir.dt.bfloat16
AF = mybir.ActivationFunctionType
ALU = mybir.AluOpType


def _body(nc, tc, aps, phases=("qkv", "attn", "wout")):
    xt, wq, wk, wv, wo, cc, ss, mk, out = aps
    with (
        tc.tile_pool(name="const", bufs=1) as const,
        tc.tile_pool(name="xtp", bufs=2) as xtp,
        tc.tile_pool(name="qkv", bufs=1) as qkvp,
        tc.tile_pool(name="rope", bufs=1) as ropep,
        tc.tile_pool(name="esb", bufs=3) as esbp,
        tc.tile_pool(name="recp", bufs=1) as recp,
        tc.tile_pool(name="bcp", bufs=2) as bcp,
        tc.tile_pool(name="attn", bufs=2) as attnp,
        tc.tile_pool(name="outp", bufs=4) as outp,
        tc.tile_pool(name="psA", bufs=2, space="PSUM") as psA,
        tc.tile_pool(name="psO", bufs=2, space="PSUM") as psO,
        tc.tile_pool(name="psS", bufs=2, space="PSUM") as psS,
    ):
        # ---- constants ----
        wq_sb = const.tile([128, KD, HPC * DH], BF16, name="wq_sb")
        wk_sb = const.tile([128, KD, HPC * DH], BF16, name="wk_sb")
        wv_sb = const.tile([128, KD, HPC * DH], BF16, name="wv_sb")
        wo_sb = const.tile([128, HPC, D], BF16, name="wo_sb")
        cc_sb = const.tile([128, L], F32, name="cc_sb")  # [cos; cos]
        ss_sb = const.tile([128, L], F32, name="ss_sb")  # [-sin; +sin]
        mk_sb = const.tile([128, 4, QC], BF16, name="mk_sb")

        def load_chunk(c):
            c0 = c * TOKC
            xtile = xtp.tile([128, KD, TOKC], BF16, name="xtile")
            src = xt[:, c0:c0 + TOKC].rearrange("(ko p) n -> p ko n", p=128)
            # two half-loads: matmuls on k<8 start as soon as half 0 lands
            nc.sync.dma_start(xtile[:, 0:KD // 2], src[:, 0:KD // 2])
            nc.sync.dma_start(xtile[:, KD // 2:], src[:, KD // 2:])
            return xtile

        # first x chunk + first weight ahead of everything else; wo last
        xtile0 = load_chunk(0) if "qkv" in phases else None
        for dst, src in ((wq_sb, wq), (cc_sb, cc), (ss_sb, ss), (mk_sb, mk),
                         (wk_sb, wk), (wv_sb, wv), (wo_sb, wo)):
            nc.sync.dma_start(dst, src)
        ones_f32 = const.tile([128, 1], F32, name="ones_f32")
        nc.vector.memset(ones_f32, 1.0)
        ones_col = const.tile([128, 1], BF16, name="ones_col")
        nc.vector.tensor_copy(ones_col, ones_f32)

        qrot = [qkvp.tile([128, L], BF16, name=f"qrot{h}") for h in range(HPC)]
        krot = [qkvp.tile([128, L], BF16, name=f"krot{h}") for h in range(HPC)]
        vnat = qkvp.tile([128, KT, HPC * DH], BF16, name="vnat")

        def proj_chunk(c):
            c0 = c * TOKC
            xtile = xtile0 if c == 0 else load_chunk(c)
            cseg = cc_sb[:, c0:c0 + TOKC]
            sseg = ss_sb[:, c0:c0 + TOKC]
            for w_sb, dsts in ((wq_sb, qrot), (wk_sb, krot)):
                for h in range(HPC):
                    ps = psA.tile([128, TOKC], F32, name="ps_proj", tag="psA")
                    for k in range(KD):
                        nc.tensor.matmul(
                            ps, w_sb[:, k, h * DH:(h + 1) * DH], xtile[:, k, :],
                            start=(k == 0), stop=(k == KD - 1),
                        )
                    if "nocopy" in phases:
                        continue
                    # RoPE copyback: dst = ps*C + swap(ps)*S
                    t = ropep.tile([128, TOKC], F32, name="rope_t")
                    a = ropep.tile([128, TOKC], F32, name="rope_a")
                    nc.vector.tensor_tensor(t[0:64], ps[64:128], sseg[0:64], ALU.mult)
                    nc.vector.tensor_tensor(t[64:128], ps[0:64], sseg[64:128], ALU.mult)
                    nc.vector.tensor_tensor(a, ps, cseg, ALU.mult)
                    nc.vector.tensor_tensor(
                        dsts[h][:, c0:c0 + TOKC], a, t, ALU.add)
            # V in natural [tok, dh] layout: x token slices stationary, all
            # 4 heads share one 512-wide moving pass
            for s in range(TOKC // 128):
                psv = psA.tile([128, HPC * DH], F32, name="ps_v", tag="psA")
                for k in range(KD):
                    nc.tensor.matmul(
                        psv, xtile[:, k, s * 128:(s + 1) * 128], wv_sb[:, k, :],
                        start=(k == 0), stop=(k == KD - 1),
                    )
                if "nocopy" in phases:
                    continue
                kti = (c0 // 128) + s
                nc.scalar.copy(vnat[:, kti, :], psv)

        def emit_wout(attn_sb, q0):
            for mt in range(QC // 128):
                t0 = q0 + mt * 128
                for ec in range(D // 512):
                    psw = psA.tile([128, 512], F32, name="ps_w", tag="psA")
                    for h in range(HPC):
                        nc.tensor.matmul(
                            psw, attn_sb[h][:, mt * 128:(mt + 1) * 128],
                            wo_sb[:, h, ec * 512:(ec + 1) * 512],
                            start=(h == 0), stop=(h == HPC - 1),
                        )
                    ob = outp.tile([128, 512], BF16, name="out_sb")
                    if (mt + ec) % 2 == 0:
                        nc.scalar.copy(ob, psw)
                    else:
                        nc.vector.tensor_copy(ob, psw)
                    eng = nc.scalar if (mt + ec) % 2 == 0 else nc.sync
                    eng.dma_start(
                        out[t0:t0 + 128, ec * 512:(ec + 1) * 512], ob)

        def attn_chunk(qc):
            q0 = qc * QC
            attn_sb = []
            for h in range(HPC):
                pso = psO.tile([128, QC], F32, name="ps_out")
                pss = psS.tile([1, QC], F32, name="ps_sum")
                nkt = (qc + 1) * (QC // 128)
                npair = nkt // 2
                epairs = [None] * npair

                def emit_attnout(q):
                    e = epairs[q]
                    for half in range(2):
                        kt = 2 * q + half
                        off = 128 * max(kt - 4 * qc, 0)  # causal shrink
                        eh = e[:, half * QC + off:(half + 1) * QC]
                        nc.tensor.matmul(
                            pso[:, off:QC], vnat[:, kt, h * DH:(h + 1) * DH],
                            eh, start=(kt == 0), stop=(kt == nkt - 1))
                        if "nosum" not in phases:
                            nc.tensor.matmul(
                                pss[:, off:QC], ones_col, eh,
                                start=(kt == 0), stop=(kt == nkt - 1))

                for p in range(npair + 2):
                    if p < npair:
                        pp = psA.tile([128, 2 * QC], F32, name="ps_sc", tag="psA")
                        for half in range(2):
                            kt = 2 * p + half
                            off = 128 * max(kt - 4 * qc, 0)  # causal shrink
                            nc.tensor.matmul(
                                pp[:, half * QC + off:(half + 1) * QC],
                                krot[h][:, kt * 128:(kt + 1) * 128],
                                qrot[h][:, q0 + off:q0 + QC],
                                start=True, stop=True,
                            )
                        e = esbp.tile([128, 2 * QC], BF16, name="e_sb")
                        if "noexp" in phases:
                            nc.scalar.copy(e, pp)
                        else:
                            nc.scalar.activation(e, pp, AF.Exp, scale=SCALE)
                        if "nomask" not in phases:
                            for half in range(2):
                                if 2 * p + half - 4 * qc >= 0:
                                    off = (half * QC
                                           + 128 * (2 * p + half - 4 * qc))
                                    eh = e[:, off:off + 128]
                                    nc.vector.tensor_tensor(
                                        eh, eh, mk_sb[:, 0, 0:128], ALU.mult)
                        epairs[p] = e
                    if p >= 2:
                        emit_attnout(p - 2)

                att = attnp.tile([128, QC], BF16, name=f"att{h}")
                if "nosum" in phases:
                    nc.vector.tensor_copy(att, pso)
                else:
                    rec = recp.tile([1, QC], F32, name="recip")
                    nc.vector.reciprocal_approx_fast(rec, pss)
                    bc = bcp.tile([128, QC], F32, name="bc_sb")
                    nc.gpsimd.partition_broadcast(bc, rec)
                    nc.vector.tensor_tensor(att, pso, bc, ALU.mult)
                attn_sb.append(att)
            return attn_sb

        pend = None
        for c in range(NCHUNK):
            if "qkv" in phases:
                proj_chunk(c)
            if "attn" in phases:
                attn_sb = attn_chunk(c)
                if pend is not None and "wout" in phases:
                    emit_wout(*pend)
                pend = (attn_sb, c * QC)
        if pend is not None and "wout" in phases:
            emit_wout(*pend)


def build_kernel(timing=False, loop_n=0, phases=("qkv", "attn", "wout")):
    nc = bacc.Bacc(
        "TRN2",
        target_bir_lowering=False,
        debug=False,
        enable_asserts=False,
        num_devices=NCORES,
    )
    xt = nc.dram_tensor("xt", [D, L], BF16, kind="ExternalInput").ap()
    wq = nc.dram_tensor("wq", [128, KD, HPC * DH], BF16, kind="ExternalInput").ap()
    wk = nc.dram_tensor("wk", [128, KD, HPC * DH], BF16, kind="ExternalInput").ap()
    wv = nc.dram_tensor("wv", [128, KD, HPC * DH], BF16, kind="ExternalInput").ap()
    wo = nc.dram_tensor("wo", [128, HPC, D], BF16, kind="ExternalInput").ap()
    cc = nc.dram_tensor("cc", [128, L], F32, kind="ExternalInput").ap()
    ss = nc.dram_tensor("ss", [128, L], F32, kind="ExternalInput").ap()
    mk = nc.dram_tensor("mk", [128, 4, QC], BF16, kind="ExternalInput").ap()
    out_kind = "Internal" if timing else "ExternalOutput"
    out = nc.dram_tensor("out", [L, D], BF16, kind=out_kind).ap()
    done = None
    if timing:
        done = nc.dram_tensor("done", [1, 4], BF16, kind="ExternalOutput").ap()

    nc.gpsimd.load_library(library_config.attn)
    aps = (xt, wq, wk, wv, wo, cc, ss, mk, out)
    with tile.TileContext(nc) as tc:
        if loop_n:
            with tc.For_i(0, loop_n, 1):
                _body(nc, tc, aps, phases)
        else:
            _body(nc, tc, aps, phases)
        if timing:
            # tiny output so the executable has an ExternalOutput; depends on
            # one real out tile via a DRAM->DRAM DMA of the last row.
            nc.sync.dma_start(done, out[L - 1:L, 0:4])
    nc.compile()
    return nc


def _rope_tables():
    inv_freq = (1.0 / (ROPE_BASE ** (np.arange(0, DH, 2, dtype=np.float32) / DH))
                ).astype(np.float32)
    freqs = (np.arange(L, dtype=np.float32)[:, None] * inv_freq[None, :]
             ).astype(np.float32)  # [L, 64]
    cos_t = np.cos(freqs).astype(np.float32).T  # [64, L]
    sin_t = np.sin(freqs).astype(np.float32).T
    cc = np.concatenate([cos_t, cos_t], axis=0)  # [128, L]
    ss = np.concatenate([-sin_t, sin_t], axis=0)
    return np.ascontiguousarray(cc), np.ascontiguousarray(ss)


def _host_inputs(x, w_qkv, w_out):
    bf16 = mybir.dt.np(BF16)
    cc, ss = _rope_tables()
    p = np.arange(128)[:, None]
    f = np.arange(QC)[None, :]
    mk = np.stack(
        [((bi * 128 + p) <= f).astype(np.float32) for bi in range(4)], axis=1
    ).astype(bf16)  # [128, 4, 512]
    mk = np.ascontiguousarray(mk)

    def wtile(wT):  # [D, M] f32 -> [128, D//128, M] bf16
        return np.ascontiguousarray(
            wT.reshape(KD, 128, wT.shape[1]).transpose(1, 0, 2)).astype(bf16)

    xts = [
        np.ascontiguousarray(x[b].T).astype(bf16)  # [D, L]
        for b in range(B)
    ]
    in_maps = []
    for c in range(NCORES):
        b, g = divmod(c, NGRP)
        r0 = g * HPC * DH
        r1 = r0 + HPC * DH
        wq_c = wtile(np.ascontiguousarray(w_qkv[r0:r1, :].T))
        wk_c = wtile(np.ascontiguousarray(w_qkv[D + r0:D + r1, :].T))
        wv_c = wtile(np.ascontiguousarray(w_qkv[2 * D + r0:2 * D + r1, :].T))
        wo_c = np.ascontiguousarray(
            w_out[:, r0:r1].T.reshape(HPC, 128, D).transpose(1, 0, 2)
        ).astype(bf16)
        in_maps.append({
            "xt": xts[b], "wq": wq_c, "wk": wk_c, "wv": wv_c, "wo": wo_c,
            "cc": cc, "ss": ss, "mk": mk,
        })
    return in_maps


_NC_CACHE = []


def _get_nc():
    if not _NC_CACHE:
        _NC_CACHE.append(build_kernel())
    return _NC_CACHE[0]


def kernel(x, w_qkv, w_out):
    x = np.asarray(x, dtype=np.float32)
    w_qkv = np.asarray(w_qkv, dtype=np.float32)
    w_out = np.asarray(w_out, dtype=np.float32)
    nc = _get_nc()
    in_maps = _host_inputs(x, w_qkv, w_out)
    res = run_bass_kernel_spmd(nc, in_maps, core_ids=list(range(NCORES)))
    out = np.zeros((B, L, D), dtype=np.float32)
    for c in range(NCORES):
        out[c // NGRP] += res.results[c]["out"].astype(np.float32)
    return out


# revision 10
# speedup vs baseline: 1.1471x; 1.0049x over previous
"""Causal self-attention with RoPE on 8 NeuronCores.

Problem shapes: x [2, 2048, 2048], w_qkv [6144, 2048], w_out [2048, 2048],
H=16 heads, dh=128.

Sharding: data-parallel over batch x tensor-parallel over heads.  Core c
owns batch c//4 and heads {4g..4g+3} with g = c%4.  Each core computes a
[2048, 2048] partial of its batch's output (w_out rows for its heads);
the host sums 4 partials per batch.  Per-core DMA: x 8.4MB bf16 in,
weights 8MB bf16, out 16.8MB f32 -- about half the all-TP layout.

All matmuls run in bf16 (1 row/cycle on PE, f32 PSUM accumulation;
rel err ~0.5% << the 2e-2 gate).  Per-core plan:

  - Q^T/K^T projection: psum[dh, tok] = w-tile (stationary) x xT (moving,
    512-token chunks); V projected into natural [tok, dh] layout by
    flipping operands (x token-slices stationary, wv moving, all 4 heads
    in one 512-wide moving pass).
  - RoPE during PSUM->SBUF copyback, 4 DVE ops per tile using
    sign-folded tables (S = [-sin; +sin], C = [cos; cos]):
    t = swap(ps) * S (2 half ops), dst = ps * C + t (2 full ops).
  - Attention per 512-token q chunk, k tiles processed in PAIRS sharing
    one [128, 1024] PSUM tile so a single exp activation covers both
    (ACT per pair 1147ns < PE per pair ~1280ns).  The attn-out/denom
    matmuls for pair p are emitted after the scores matmuls of pair p+2
    (software pipelining, 2-pair lag) so PE never waits on ACT.
  - Causal mask: multiply e by 0/1 tiles on diagonal blocks (bf16 DVE).
  - Softmax denominator accumulates ones^T x e on PE alongside attn-out;
    reciprocal via the 1-instruction approx-fast DVE op (~51 ULP),
    broadcast across partitions with gpsimd, multiplied in during the
    attn-out copyback.
  - w_out for chunk qc is emitted after attention for qc+1; projections
    for chunk c+1 interleave with attention for chunk c.
"""

import numpy as np

import concourse.bass as bass
import concourse.mybir as mybir
import concourse.tile as tile
from concourse import bacc, library_config
from concourse.bass_utils import run_bass_kernel_spmd

B, L, D, H = 2, 2048, 2048, 16
DH = D // H  # 128
NCORES = 8
NGRP = 4  # head groups (cores per batch)
HPC = H // NGRP  # 4 heads per core
ROPE_BASE = 10000.0
SCALE = 1.0 / float(np.sqrt(np.float32(DH)))

TOKC = 512  # token chunk width in the QKV projection phase
NCHUNK = L // TOKC  # 4
QC = 512  # q chunk width in the attention phase (== TOKC)
KT = L // 128  # 16 k tiles per sequence
KD = D // 128  # 16 contraction chunks for the projections

F32 = mybir.dt.float32
BF16 = mybir.dt.bfloat16
AF = mybir.ActivationFunctionType
ALU = mybir.AluOpType


def _body(nc, tc, aps, phases=("qkv", "attn", "wout")):
    xt, wq, wk, wv, wo, cc, ss, mk, out = aps
    with (
        tc.tile_pool(name="const", bufs=1) as const,
        tc.tile_pool(name="xtp", bufs=2) as xtp,
        tc.tile_pool(name="qkv", bufs=1) as qkvp,
        tc.tile_pool(name="rope", bufs=1) as ropep,
        tc.tile_pool(name="esb", bufs=4) as esbp,
        tc.tile_pool(name="recp", bufs=1) as recp,
        tc.tile_pool(name="bcp", bufs=2) as bcp,
        tc.tile_pool(name="attn", bufs=2) as attnp,
        tc.tile_pool(name="outp", bufs=6) as outp,
        tc.tile_pool(name="psA", bufs=2, space="PSUM") as psA,
        tc.tile_pool(name="psO", bufs=2, space="PSUM") as psO,
        tc.tile_pool(name="psS", bufs=2, space="PSUM") as psS,
    ):
        # ---- constants ----
        wq_sb = const.tile([128, KD, HPC * DH], BF16, name="wq_sb")
        wk_sb = const.tile([128, KD, HPC * DH], BF16, name="wk_sb")
        wv_sb = const.tile([128, KD, HPC * DH], BF16, name="wv_sb")
        wo_sb = const.tile([128, HPC, D], BF16, name="wo_sb")
        cc_sb = const.tile([128, L], F32, name="cc_sb")  # [cos; cos]
        ss_sb = const.tile([128, L], F32, name="ss_sb")  # [-sin; +sin]
        mk_sb = const.tile([128, 4, QC], BF16, name="mk_sb")

        def load_chunk(c):
            c0 = c * TOKC
            xtile = xtp.tile([128, KD, TOKC], BF16, name="xtile")
            src = xt[:, c0:c0 + TOKC].rearrange("(ko p) n -> p ko n", p=128)
            # two half-loads: matmuls on k<8 start as soon as half 0 lands
            nc.sync.dma_start(xtile[:, 0:KD // 2], src[:, 0:KD // 2])
            nc.sync.dma_start(xtile[:, KD // 2:], src[:, KD // 2:])
            return xtile

        # first x chunk + first weight ahead of everything else; wo last
        xtile0 = load_chunk(0) if "qkv" in phases else None
        for dst, src in ((wq_sb, wq), (cc_sb, cc), (ss_sb, ss), (mk_sb, mk),
                         (wk_sb, wk), (wv_sb, wv), (wo_sb, wo)):
            nc.sync.dma_start(dst, src)
        ones_f32 = const.tile([128, 1], F32, name="ones_f32")
        nc.vector.memset(ones_f32, 1.0)
        ones_col = const.tile([128, 1], BF16, name="ones_col")
        nc.vector.tensor_copy(ones_col, ones_f32)

        qrot = [qkvp.tile([128, L], BF16, name=f"qrot{h}") for h in range(HPC)]
        krot = [qkvp.tile([128, L], BF16, name=f"krot{h}") for h in range(HPC)]
        vnat = qkvp.tile([128, KT, HPC * DH], BF16, name="vnat")

        def proj_chunk(c):
            c0 = c * TOKC
            xtile = xtile0 if c == 0 else load_chunk(c)
            cseg = cc_sb[:, c0:c0 + TOKC]
            sseg = ss_sb[:, c0:c0 + TOKC]
            for w_sb, dsts in ((wq_sb, qrot), (wk_sb, krot)):
                for h in range(HPC):
                    ps = psA.tile([128, TOKC], F32, name="ps_proj", tag="psA")
                    for k in range(KD):
                        nc.tensor.matmul(
                            ps, w_sb[:, k, h * DH:(h + 1) * DH], xtile[:, k, :],
                            start=(k == 0), stop=(k == KD - 1),
                        )
                    if "nocopy" in phases:
                        continue
                    # RoPE copyback: dst = ps*C + swap(ps)*S
                    t = ropep.tile([128, TOKC], F32, name="rope_t")
                    a = ropep.tile([128, TOKC], F32, name="rope_a")
                    nc.vector.tensor_tensor(t[0:64], ps[64:128], sseg[0:64], ALU.mult)
                    nc.vector.tensor_tensor(t[64:128], ps[0:64], sseg[64:128], ALU.mult)
                    nc.vector.tensor_tensor(a, ps, cseg, ALU.mult)
                    nc.vector.tensor_tensor(
                        dsts[h][:, c0:c0 + TOKC], a, t, ALU.add)
            # V in natural [tok, dh] layout: x token slices stationary, all
            # 4 heads share one 512-wide moving pass
            for s in range(TOKC // 128):
                psv = psA.tile([128, HPC * DH], F32, name="ps_v", tag="psA")
                for k in range(KD):
                    nc.tensor.matmul(
                        psv, xtile[:, k, s * 128:(s + 1) * 128], wv_sb[:, k, :],
                        start=(k == 0), stop=(k == KD - 1),
                    )
                if "nocopy" in phases:
                    continue
                kti = (c0 // 128) + s
                nc.scalar.copy(vnat[:, kti, :], psv)

        def emit_wout(attn_sb, q0):
            for mt in range(QC // 128):
                t0 = q0 + mt * 128
                for ec in range(D // 512):
                    psw = psA.tile([128, 512], F32, name="ps_w", tag="psA")
                    for h in range(HPC):
                        nc.tensor.matmul(
                            psw, attn_sb[h][:, mt * 128:(mt + 1) * 128],
                            wo_sb[:, h, ec * 512:(ec + 1) * 512],
                            start=(h == 0), stop=(h == HPC - 1),
                        )
                    ob = outp.tile([128, 512], BF16, name="out_sb")
                    if (mt + ec) % 2 == 0:
                        nc.scalar.copy(ob, psw)
                    else:
                        nc.vector.tensor_copy(ob, psw)
                    eng = nc.scalar if (mt + ec) % 2 == 0 else nc.sync
                    eng.dma_start(
                        out[t0:t0 + 128, ec * 512:(ec + 1) * 512], ob)

        def attn_chunk(qc):
            q0 = qc * QC
            attn_sb = []
            for h in range(HPC):
                pso = psO.tile([128, QC], F32, name="ps_out")
                pss = psS.tile([1, QC], F32, name="ps_sum")
                nkt = (qc + 1) * (QC // 128)
                npair = nkt // 2
                epairs = [None] * npair

                def emit_attnout(q):
                    e = epairs[q]
                    for half in range(2):
                        kt = 2 * q + half
                        off = 128 * max(kt - 4 * qc, 0)  # causal shrink
                        eh = e[:, half * QC + off:(half + 1) * QC]
                        nc.tensor.matmul(
                            pso[:, off:QC], vnat[:, kt, h * DH:(h + 1) * DH],
                            eh, start=(kt == 0), stop=(kt == nkt - 1))
                        if "nosum" not in phases:
                            nc.tensor.matmul(
                                pss[:, off:QC], ones_col, eh,
                                start=(kt == 0), stop=(kt == nkt - 1))

                for p in range(npair + 2):
                    if p < npair:
                        pp = psA.tile([128, 2 * QC], F32, name="ps_sc", tag="psA")
                        for half in range(2):
                            kt = 2 * p + half
                            off = 128 * max(kt - 4 * qc, 0)  # causal shrink
                            nc.tensor.matmul(
                                pp[:, half * QC + off:(half + 1) * QC],
                                krot[h][:, kt * 128:(kt + 1) * 128],
                                qrot[h][:, q0 + off:q0 + QC],
                                start=True, stop=True,
                            )
                        e = esbp.tile([128, 2 * QC], BF16, name="e_sb")
                        if "noexp" in phases:
                            nc.scalar.copy(e, pp)
                        else:
                            nc.scalar.activation(e, pp, AF.Exp, scale=SCALE)
                        if "nomask" not in phases:
                            for half in range(2):
                                if 2 * p + half - 4 * qc >= 0:
                                    off = (half * QC
                                           + 128 * (2 * p + half - 4 * qc))
                                    eh = e[:, off:off + 128]
                                    nc.vector.tensor_tensor(
                                        eh, eh, mk_sb[:, 0, 0:128], ALU.mult)
                        epairs[p] = e
                    if p >= 2:
                        emit_attnout(p - 2)

                att = attnp.tile([128, QC], BF16, name=f"att{h}")
                if "nosum" in phases:
                    nc.vector.tensor_copy(att, pso)
                else:
                    rec = recp.tile([1, QC], F32, name="recip")
                    nc.vector.reciprocal_approx_fast(rec, pss)
                    bc = bcp.tile([128, QC], F32, name="bc_sb")
                    nc.gpsimd.partition_broadcast(bc, rec)
                    nc.vector.tensor_tensor(att, pso, bc, ALU.mult)
                attn_sb.append(att)
            return attn_sb

        pend = None
        for c in range(NCHUNK):
            if "qkv" in phases:
                proj_chunk(c)
            if "attn" in phases:
                attn_sb = attn_chunk(c)
                if pend is not None and "wout" in phases:
                    emit_wout(*pend)
                pend = (attn_sb, c * QC)
        if pend is not None and "wout" in phases:
            emit_wout(*pend)


def build_kernel(timing=False, loop_n=0, phases=("qkv", "attn", "wout")):
    nc = bacc.Bacc(
        "TRN2",
        target_bir_lowering=False,
        debug=False,
        enable_asserts=False,
        num_devices=NCORES,
    )
    xt = nc.dram_tensor("xt", [D, L], BF16, kind="ExternalInput").ap()
    wq = nc.dram_tensor("wq", [128, KD, HPC * DH], BF16, kind="ExternalInput").ap()
    wk = nc.dram_tensor("wk", [128, KD, HPC * DH], BF16, kind="ExternalInput").ap()
    wv = nc.dram_tensor("wv", [128, KD, HPC * DH], BF16, kind="ExternalInput").ap()
    wo = nc.dram_tensor("wo", [128, HPC, D], BF16, kind="ExternalInput").ap()
    cc = nc.dram_tensor("cc", [128, L], F32, kind="ExternalInput").ap()
    ss = nc.dram_tensor("ss", [128, L], F32, kind="ExternalInput").ap()
    mk = nc.dram_tensor("mk", [128, 4, QC], BF16, kind="ExternalInput").ap()
    out_kind = "Internal" if timing else "ExternalOutput"
    out = nc.dram_tensor("out", [L, D], BF16, kind=out_kind).ap()
    done = None
    if timing:
        done = nc.dram_tensor("done", [1, 4], BF16, kind="ExternalOutput").ap()

    nc.gpsimd.load_library(library_config.attn)
    aps = (xt, wq, wk, wv, wo, cc, ss, mk, out)
    with tile.TileContext(nc) as tc:
        if loop_n:
            with tc.For_i(0, loop_n, 1):
                _body(nc, tc, aps, phases)
        else:
            _body(nc, tc, aps, phases)
        if timing:
            # tiny output so the executable has an ExternalOutput; depends on
            # one real out tile via a DRAM->DRAM DMA of the last row.
            nc.sync.dma_start(done, out[L - 1:L, 0:4])
    nc.compile()
    return nc


def _rope_tables():
    inv_freq = (1.0 / (ROPE_BASE ** (np.arange(0, DH, 2, dtype=np.float32) / DH))
                ).astype(np.float32)
    freqs = (np.arange(L, dtype=np.float32)[:, None] * inv_freq[None, :]
             ).astype(np.float32)  # [L, 64]
    cos_t = np.cos(freqs).astype(np.float32).T  # [64, L]
    sin_t = np.sin(freqs).astype(np.float32).T
    cc = np.concatenate([cos_t, cos_t], axis=0)  # [128, L]
    ss = np.concatenate([-sin_t, sin_t], axis=0)
    return np.ascontiguousarray(cc), np.ascontiguousarray(ss)


def _host_inputs(x, w_qkv, w_out):
    bf16 = mybir.dt.np(BF16)
    cc, ss = _rope_tables()
    p = np.arange(128)[:, None]
    f = np.arange(QC)[None, :]
    mk = np.stack(
        [((bi * 128 + p) <= f).astype(np.float32) for bi in range(4)], axis=1
    ).astype(bf16)  # [128, 4, 512]
    mk = np.ascontiguousarray(mk)

    def wtile(wT):  # [D, M] f32 -> [128, D//128, M] bf16
        return np.ascontiguousarray(
            wT.reshape(KD, 128, wT.shape[1]).transpose(1, 0, 2)).astype(bf16)

    xts = [
        np.ascontiguousarray(x[b].T).astype(bf16)  # [D, L]
        for b in range(B)
    ]
    in_maps = []
    for c in range(NCORES):
        b, g = divmod(c, NGRP)
        r0 = g * HPC * DH
        r1 = r0 + HPC * DH
        wq_c = wtile(np.ascontiguousarray(w_qkv[r0:r1, :].T))
        wk_c = wtile(np.ascontiguousarray(w_qkv[D + r0:D + r1, :].T))
        wv_c = wtile(np.ascontiguousarray(w_qkv[2 * D + r0:2 * D + r1, :].T))
        wo_c = np.ascontiguousarray(
            w_out[:, r0:r1].T.reshape(HPC, 128, D).transpose(1, 0, 2)
        ).astype(bf16)
        in_maps.append({
            "xt": xts[b], "wq": wq_c, "wk": wk_c, "wv": wv_c, "wo": wo_c,
            "cc": cc, "ss": ss, "mk": mk,
        })
    return in_maps


_NC_CACHE = []


def _get_nc():
    if not _NC_CACHE:
        _NC_CACHE.append(build_kernel())
    return _NC_CACHE[0]


def kernel(x, w_qkv, w_out):
    x = np.asarray(x, dtype=np.float32)
    w_qkv = np.asarray(w_qkv, dtype=np.float32)
    w_out = np.asarray(w_out, dtype=np.float32)
    nc = _get_nc()
    in_maps = _host_inputs(x, w_qkv, w_out)
    res = run_bass_kernel_spmd(nc, in_maps, core_ids=list(range(NCORES)))
    out = np.zeros((B, L, D), dtype=np.float32)
    for c in range(NCORES):
        out[c // NGRP] += res.results[c]["out"].astype(np.float32)
    return out


# revision 15
# speedup vs baseline: 1.2378x; 1.0791x over previous
"""Causal self-attention with RoPE on 8 NeuronCores.

Problem shapes: x [2, 2048, 2048], w_qkv [6144, 2048], w_out [2048, 2048],
H=16 heads, dh=128.

Sharding: data-parallel over batch x tensor-parallel over heads.  Core c
owns batch c//4 and heads {4g..4g+3} with g = c%4.  Each core computes a
[2048, 2048] partial of its batch's output (w_out rows for its heads);
the host sums 4 partials per batch.  Per-core DMA: x 8.4MB bf16 in,
weights 8MB bf16, out 8.4MB bf16 -- about a quarter of the all-TP/f32
layout.

All matmuls run in bf16 (1 row/cycle on PE, f32 PSUM accumulation;
measured rel err ~5e-3 vs the 2e-2 gate).  Measured cost is ~281ns per
512-wide matmul (213ns stream + ~68ns self-loading-LDWEIGHTS tax that
walrus --enable-ldw-opt=false cannot hide), so the design minimizes
matmul count and keeps every other engine off the PE critical path.
Per-core plan:

  - Q^T/K^T projection: psum[dh, tok] = w-tile (stationary) x xT (moving,
    512-token chunks); V projected into natural [tok, dh] layout by
    flipping operands (x token-slices stationary, wv moving, all 4 heads
    in one 512-wide moving pass).
  - RoPE during PSUM->SBUF copyback, 4 DVE ops per tile using
    sign-folded tables (S = [-sin; +sin], C = [cos; cos]):
    t = swap(ps) * S (2 half ops), dst = ps * C + t (2 full ops).
  - Attention per 512-token q chunk, k tiles processed in PAIRS sharing
    one [128, 1024] PSUM tile so a single exp activation covers both
    (ACT per pair 1147ns < PE per pair ~1280ns).  The attn-out/denom
    matmuls for pair p are emitted after the scores matmuls of pair p+2
    (software pipelining, 2-pair lag) so PE never waits on ACT.
  - Causality: diagonal-block matmuls (scores/attn-out/denominator) shrink
    their moving free dim to the valid q range (saves ~37% of the diagonal
    rows); the remaining triangular 128-col block of each is masked by a
    single shared 0/1 multiply (bf16 DVE).  exp runs over the full pair
    tile -- the gap regions hold stale PSUM whose exp is never read.
  - Softmax denominator accumulates ones^T x e on PE alongside attn-out;
    reciprocal via the 1-instruction approx-fast DVE op (~51 ULP),
    broadcast across partitions with gpsimd, multiplied in during the
    attn-out copyback.
  - w_out for chunk qc is emitted after attention for qc+1; projections
    for chunk c+1 interleave with attention for chunk c.
"""

import numpy as np

import concourse.bass as bass
import concourse.mybir as mybir
import concourse.tile as tile
from concourse import bacc, library_config
from concourse.bass_utils import run_bass_kernel_spmd

B, L, D, H = 2, 2048, 2048, 16
DH = D // H  # 128
NCORES = 8
NGRP = 4  # head groups (cores per batch)
HPC = H // NGRP  # 4 heads per core
ROPE_BASE = 10000.0
SCALE = 1.0 / float(np.sqrt(np.float32(DH)))

TOKC = 512  # token chunk width in the QKV projection phase
NCHUNK = L // TOKC  # 4
QC = 512  # q chunk width in the attention phase (== TOKC)
KT = L // 128  # 16 k tiles per sequence
KD = D // 128  # 16 contraction chunks for the projections

F32 = mybir.dt.float32
BF16 = mybir.dt.bfloat16
AF = mybir.ActivationFunctionType
ALU = mybir.AluOpType


def _body(nc, tc, aps, phases=("qkv", "attn", "wout")):
    xt, wq, wk, wv, wo, cc, ss, mk, out = aps
    with (
        tc.tile_pool(name="const", bufs=1) as const,
        tc.tile_pool(name="xtp", bufs=2) as xtp,
        tc.tile_pool(name="qkv", bufs=1) as qkvp,
        tc.tile_pool(name="rope", bufs=1) as ropep,
        tc.tile_pool(name="esb", bufs=4) as esbp,
        tc.tile_pool(name="esum", bufs=3) as esump,
        tc.tile_pool(name="recp", bufs=1) as recp,
        tc.tile_pool(name="bcp", bufs=2) as bcp,
        tc.tile_pool(name="attn", bufs=2) as attnp,
        tc.tile_pool(name="outp", bufs=6) as outp,
        tc.tile_pool(name="psA", bufs=2, space="PSUM") as psA,
        tc.tile_pool(name="psO", bufs=2, space="PSUM") as psO,
        tc.tile_pool(name="psS", bufs=2, space="PSUM") as psS,
    ):
        # ---- constants ----
        wq_sb = const.tile([128, KD, HPC * DH], BF16, name="wq_sb")
        wk_sb = const.tile([128, KD, HPC * DH], BF16, name="wk_sb")
        wv_sb = const.tile([128, KD, HPC * DH], BF16, name="wv_sb")
        wo_sb = const.tile([128, HPC, D], BF16, name="wo_sb")
        cc_sb = const.tile([128, L], F32, name="cc_sb")  # [cos; cos]
        ss_sb = const.tile([128, L], F32, name="ss_sb")  # [-sin; +sin]
        mk_sb = const.tile([128, 4, QC], BF16, name="mk_sb")

        def load_chunk(c):
            c0 = c * TOKC
            xtile = xtp.tile([128, KD, TOKC], BF16, name="xtile")
            src = xt[:, c0:c0 + TOKC].rearrange("(ko p) n -> p ko n", p=128)
            # two half-loads: matmuls on k<8 start as soon as half 0 lands
            nc.sync.dma_start(xtile[:, 0:KD // 2], src[:, 0:KD // 2])
            nc.sync.dma_start(xtile[:, KD // 2:], src[:, KD // 2:])
            return xtile

        # first x chunk + first weight ahead of everything else; wo last
        xtile0 = load_chunk(0) if "qkv" in phases else None
        for dst, src in ((wq_sb, wq), (cc_sb, cc), (ss_sb, ss), (mk_sb, mk),
                         (wk_sb, wk), (wv_sb, wv), (wo_sb, wo)):
            nc.sync.dma_start(dst, src)
        ones_f32 = const.tile([128, 1], F32, name="ones_f32")
        nc.vector.memset(ones_f32, 1.0)
        ones_col = const.tile([128, 1], BF16, name="ones_col")
        nc.vector.tensor_copy(ones_col, ones_f32)

        qrot = [qkvp.tile([128, L], BF16, name=f"qrot{h}") for h in range(HPC)]
        krot = [qkvp.tile([128, L], BF16, name=f"krot{h}") for h in range(HPC)]
        vnat = qkvp.tile([128, KT, HPC * DH], BF16, name="vnat")

        def proj_chunk(c):
            c0 = c * TOKC
            xtile = xtile0 if c == 0 else load_chunk(c)
            cseg = cc_sb[:, c0:c0 + TOKC]
            sseg = ss_sb[:, c0:c0 + TOKC]
            for w_sb, dsts in ((wq_sb, qrot), (wk_sb, krot)):
                for h in range(HPC):
                    ps = psA.tile([128, TOKC], F32, name="ps_proj", tag="psA")
                    for k in range(KD):
                        nc.tensor.matmul(
                            ps, w_sb[:, k, h * DH:(h + 1) * DH], xtile[:, k, :],
                            start=(k == 0), stop=(k == KD - 1),
                        )
                    if "nocopy" in phases:
                        continue
                    # RoPE copyback: dst = ps*C + swap(ps)*S
                    t = ropep.tile([128, TOKC], F32, name="rope_t")
                    a = ropep.tile([128, TOKC], F32, name="rope_a")
                    nc.vector.tensor_tensor(t[0:64], ps[64:128], sseg[0:64], ALU.mult)
                    nc.vector.tensor_tensor(t[64:128], ps[0:64], sseg[64:128], ALU.mult)
                    nc.vector.tensor_tensor(a, ps, cseg, ALU.mult)
                    nc.vector.tensor_tensor(
                        dsts[h][:, c0:c0 + TOKC], a, t, ALU.add)
            # V in natural [tok, dh] layout: x token slices stationary, all
            # 4 heads share one 512-wide moving pass
            for s in range(TOKC // 128):
                psv = psA.tile([128, HPC * DH], F32, name="ps_v", tag="psA")
                for k in range(KD):
                    nc.tensor.matmul(
                        psv, xtile[:, k, s * 128:(s + 1) * 128], wv_sb[:, k, :],
                        start=(k == 0), stop=(k == KD - 1),
                    )
                if "nocopy" in phases:
                    continue
                kti = (c0 // 128) + s
                nc.scalar.copy(vnat[:, kti, :], psv)

        def emit_wout(attn_sb, q0):
            for mt in range(QC // 128):
                t0 = q0 + mt * 128
                for ec in range(D // 512):
                    psw = psA.tile([128, 512], F32, name="ps_w", tag="psA")
                    for h in range(HPC):
                        nc.tensor.matmul(
                            psw, attn_sb[h][:, mt * 128:(mt + 1) * 128],
                            wo_sb[:, h, ec * 512:(ec + 1) * 512],
                            start=(h == 0), stop=(h == HPC - 1),
                        )
                    ob = outp.tile([128, 512], BF16, name="out_sb")
                    if (mt + ec) % 2 == 0:
                        nc.scalar.copy(ob, psw)
                    else:
                        nc.vector.tensor_copy(ob, psw)
                    eng = nc.scalar if (mt + ec) % 2 == 0 else nc.sync
                    eng.dma_start(
                        out[t0:t0 + 128, ec * 512:(ec + 1) * 512], ob)

        def attn_chunk(qc):
            q0 = qc * QC
            attn_sb = []
            for h in range(HPC):
                pso = psO.tile([128, QC], F32, name="ps_out")
                pss = psS.tile([1, QC], F32, name="ps_sum")
                nkt = (qc + 1) * (QC // 128)
                npair = nkt // 2
                epairs = [None] * npair
                esums = [None] * npair

                def emit_attnout(q):
                    e = epairs[q]
                    for half in range(2):
                        kt = 2 * q + half
                        off = 128 * max(kt - 4 * qc, 0)  # causal shrink
                        eh = e[:, half * QC + off:(half + 1) * QC]
                        nc.tensor.matmul(
                            pso[:, off:QC], vnat[:, kt, h * DH:(h + 1) * DH],
                            eh, start=(kt == 0), stop=(kt == nkt - 1))
                    if "nosum" not in phases:
                        # one denominator matmul per PAIR over the
                        # DVE-presummed e0+e1 (halves the ones-matmul count)
                        off0 = 128 * max(2 * q - 4 * qc, 0)
                        nc.tensor.matmul(
                            pss[:, off0:QC], ones_col, esums[q][:, off0:QC],
                            start=(q == 0), stop=(q == npair - 1))

                for p in range(npair + 2):
                    if p < npair:
                        pp = psA.tile([128, 2 * QC], F32, name="ps_sc", tag="psA")
                        for half in range(2):
                            kt = 2 * p + half
                            off = 128 * max(kt - 4 * qc, 0)  # causal shrink
                            nc.tensor.matmul(
                                pp[:, half * QC + off:(half + 1) * QC],
                                krot[h][:, kt * 128:(kt + 1) * 128],
                                qrot[h][:, q0 + off:q0 + QC],
                                start=True, stop=True,
                            )
                        e = esbp.tile([128, 2 * QC], BF16, name="e_sb")
                        if "noexp" in phases:
                            nc.scalar.copy(e, pp)
                        else:
                            nc.scalar.activation(e, pp, AF.Exp, scale=SCALE)
                        if "nomask" not in phases:
                            for half in range(2):
                                if 2 * p + half - 4 * qc >= 0:
                                    off = (half * QC
                                           + 128 * (2 * p + half - 4 * qc))
                                    eh = e[:, off:off + 128]
                                    nc.vector.tensor_tensor(
                                        eh, eh, mk_sb[:, 0, 0:128], ALU.mult)
                        epairs[p] = e
                        if "nosum" not in phases:
                            es = esump.tile([128, QC], BF16, name="esum")
                            d0 = 2 * p - 4 * qc
                            if d0 >= 0:  # diagonal pair: halves differ by 128
                                o = 128 * d0
                                nc.vector.tensor_copy(
                                    es[:, o:o + 128], e[:, o:o + 128])
                                nc.vector.tensor_tensor(
                                    es[:, o + 128:QC], e[:, o + 128:QC],
                                    e[:, QC + o + 128:2 * QC], ALU.add)
                            else:
                                nc.vector.tensor_tensor(
                                    es[:, 0:QC], e[:, 0:QC], e[:, QC:2 * QC],
                                    ALU.add)
                            esums[p] = es
                    if p >= 2:
                        emit_attnout(p - 2)

                att = attnp.tile([128, QC], BF16, name=f"att{h}")
                if "nosum" in phases:
                    nc.vector.tensor_copy(att, pso)
                else:
                    rec = recp.tile([1, QC], F32, name="recip")
                    nc.vector.reciprocal_approx_fast(rec, pss)
                    bc = bcp.tile([128, QC], F32, name="bc_sb")
                    nc.gpsimd.partition_broadcast(bc, rec)
                    nc.vector.tensor_tensor(att, pso, bc, ALU.mult)
                attn_sb.append(att)
            return attn_sb

        pend = None
        for c in range(NCHUNK):
            if "qkv" in phases:
                proj_chunk(c)
            if "attn" in phases:
                attn_sb = attn_chunk(c)
                if pend is not None and "wout" in phases:
                    emit_wout(*pend)
                pend = (attn_sb, c * QC)
        if pend is not None and "wout" in phases:
            emit_wout(*pend)


def build_kernel(timing=False, loop_n=0, phases=("qkv", "attn", "wout")):
    nc = bacc.Bacc(
        "TRN2",
        target_bir_lowering=False,
        debug=False,
        enable_asserts=False,
        num_devices=NCORES,
    )
    xt = nc.dram_tensor("xt", [D, L], BF16, kind="ExternalInput").ap()
    wq = nc.dram_tensor("wq", [128, KD, HPC * DH], BF16, kind="ExternalInput").ap()
    wk = nc.dram_tensor("wk", [128, KD, HPC * DH], BF16, kind="ExternalInput").ap()
    wv = nc.dram_tensor("wv", [128, KD, HPC * DH], BF16, kind="ExternalInput").ap()
    wo = nc.dram_tensor("wo", [128, HPC, D], BF16, kind="ExternalInput").ap()
    cc = nc.dram_tensor("cc", [128, L], F32, kind="ExternalInput").ap()
    ss = nc.dram_tensor("ss", [128, L], F32, kind="ExternalInput").ap()
    mk = nc.dram_tensor("mk", [128, 4, QC], BF16, kind="ExternalInput").ap()
    out_kind = "Internal" if timing else "ExternalOutput"
    out = nc.dram_tensor("out", [L, D], BF16, kind=out_kind).ap()
    done = None
    if timing:
        done = nc.dram_tensor("done", [1, 4], BF16, kind="ExternalOutput").ap()

    nc.gpsimd.load_library(library_config.attn)
    aps = (xt, wq, wk, wv, wo, cc, ss, mk, out)
    with tile.TileContext(nc) as tc:
        if loop_n:
            with tc.For_i(0, loop_n, 1):
                _body(nc, tc, aps, phases)
        else:
            _body(nc, tc, aps, phases)
        if timing:
            # tiny output so the executable has an ExternalOutput; depends on
            # one real out tile via a DRAM->DRAM DMA of the last row.
            nc.sync.dma_start(done, out[L - 1:L, 0:4])
    nc.compile()
    return nc


def _rope_tables():
    inv_freq = (1.0 / (ROPE_BASE ** (np.arange(0, DH, 2, dtype=np.float32) / DH))
                ).astype(np.float32)
    freqs = (np.arange(L, dtype=np.float32)[:, None] * inv_freq[None, :]
             ).astype(np.float32)  # [L, 64]
    cos_t = np.cos(freqs).astype(np.float32).T  # [64, L]
    sin_t = np.sin(freqs).astype(np.float32).T
    cc = np.concatenate([cos_t, cos_t], axis=0)  # [128, L]
    ss = np.concatenate([-sin_t, sin_t], axis=0)
    return np.ascontiguousarray(cc), np.ascontiguousarray(ss)


def _host_inputs(x, w_qkv, w_out):
    bf16 = mybir.dt.np(BF16)
    cc, ss = _rope_tables()
    p = np.arange(128)[:, None]
    f = np.arange(QC)[None, :]
    mk = np.stack(
        [((bi * 128 + p) <= f).astype(np.float32) for bi in range(4)], axis=1
    ).astype(bf16)  # [128, 4, 512]
    mk = np.ascontiguousarray(mk)

    def wtile(wT):  # [D, M] f32 -> [128, D//128, M] bf16
        return np.ascontiguousarray(
            wT.reshape(KD, 128, wT.shape[1]).transpose(1, 0, 2)).astype(bf16)

    xts = [
        np.ascontiguousarray(x[b].T).astype(bf16)  # [D, L]
        for b in range(B)
    ]
    in_maps = []
    for c in range(NCORES):
        b, g = divmod(c, NGRP)
        r0 = g * HPC * DH
        r1 = r0 + HPC * DH
        wq_c = wtile(np.ascontiguousarray(w_qkv[r0:r1, :].T))
        wk_c = wtile(np.ascontiguousarray(w_qkv[D + r0:D + r1, :].T))
        wv_c = wtile(np.ascontiguousarray(w_qkv[2 * D + r0:2 * D + r1, :].T))
        wo_c = np.ascontiguousarray(
            w_out[:, r0:r1].T.reshape(HPC, 128, D).transpose(1, 0, 2)
        ).astype(bf16)
        in_maps.append({
            "xt": xts[b], "wq": wq_c, "wk": wk_c, "wv": wv_c, "wo": wo_c,
            "cc": cc, "ss": ss, "mk": mk,
        })
    return in_maps


_NC_CACHE = []


def _get_nc():
    if not _NC_CACHE:
        _NC_CACHE.append(build_kernel())
    return _NC_CACHE[0]


def kernel(x, w_qkv, w_out):
    x = np.asarray(x, dtype=np.float32)
    w_qkv = np.asarray(w_qkv, dtype=np.float32)
    w_out = np.asarray(w_out, dtype=np.float32)
    nc = _get_nc()
    in_maps = _host_inputs(x, w_qkv, w_out)
    res = run_bass_kernel_spmd(nc, in_maps, core_ids=list(range(NCORES)))
    out = np.zeros((B, L, D), dtype=np.float32)
    for c in range(NCORES):
        out[c // NGRP] += res.results[c]["out"].astype(np.float32)
    return out
